# revision 1
# baseline (speedup 1.0000x reference)
"""Trainium2 Bass kernel v2 for nn_Block: fp8 DoubleRow + split exp.

Strategy vs baseline (497us):
  - All big GEMMs in fp8e4 with DoubleRow perf mode (256-deep contraction
    at 0.5 cycles/row): QKV, S, PV, proj, MLP (MLP weights as residual
    PAIRS: W ~ f8(W*64) + f8(W*64 - f8(W*64)), both accumulated in PSUM,
    giving ~0.15% weight error at half bf16 cost).
  - Softmax exp split across ACT (exact Exp -> fp8 out) and DVE
    (Schraudolph bit-trick: int8(S*1.4427 + 55.657) bitcast to e4m3).
  - PV matmul outputs O^T [65, 512] per (head, query-block); 65th V'
    column of ones yields softmax denominators in row 64. Normalization:
    ACT copy -> DVE recip of row 64 -> Pool partition_broadcast ->
    Pool multiply into oT [64, 12, 1024] (head-major dh on 64 partitions);
    proj contracts via [64,2]-DoubleRow over head pairs.
  - Sequence sharding as baseline: core c = batch c//2, half c%2 of the
    queries; K/V computed for the full 2048 tokens redundantly per pair.
"""

import numpy as np
import ml_dtypes

import concourse.bass as bass
import concourse.tile as tile
from concourse import bacc, mybir
from concourse.bass_utils import run_bass_kernel_spmd
from concourse.masks import make_identity

F32 = mybir.dt.float32
BF16 = mybir.dt.bfloat16
F8 = mybir.dt.float8e4
I8 = mybir.dt.int8
AF = mybir.ActivationFunctionType
ALU = mybir.AluOpType
DR = mybir.MatmulPerfMode.DoubleRow
E4 = ml_dtypes.float8_e4m3

B, N, C = 4, 2048, 768
H, DH = 12, 64
HID = 4 * C
EPS = 1e-5
NCORES = 8

SEQ = N
OWN = N // 2
T16 = SEQ // 128
T8 = OWN // 128
CC = C // 128          # 6 channel chunks
KC = C // 256          # 3 DR contraction chunks
HC = HID // 128        # 24 hidden chunks
HKC = HID // 256       # 12 DR chunks of hidden
WS = 64.0              # fp8 weight scale
ISC = 1.0 / WS
SM_SCALE = float(DH) ** -0.5
EXP_A = 8.0 * np.log2(np.e) * SM_SCALE   # schraudolph multiplier on raw S
EXP_B = 56.0 - 0.343                     # schraudolph bias (rint semantics)

# exp tile assignment: of every 16 (h, ktp) exp tiles, this many go to ACT
ACT_OF_16 = 8
W2_PAIR = False
W1_PAIR = True
PIPE_LAG = 1

_CACHE = {}


def _build_program(flags, debug_dumps=False):
    has_bqk, has_bias2, has_b2, has_b1 = flags
    nc = bacc.Bacc("TRN2", target_bir_lowering=False, debug=False,
                   num_devices=NCORES)
    dbg = {}
    if debug_dumps:
        dbg["hT"] = nc.dram_tensor("d_hT", [128, CC, SEQ], F32,
                                   kind="ExternalOutput").ap()
        dbg["kT"] = nc.dram_tensor("d_kT", [128, 7, SEQ], F32,
                                   kind="ExternalOutput").ap()
        dbg["qT"] = nc.dram_tensor("d_qT", [128, 7, OWN], F32,
                                   kind="ExternalOutput").ap()
        dbg["v0"] = nc.dram_tensor("d_v0", [128, 2, H, 80], F32,
                                   kind="ExternalOutput").ap()
        dbg["p00"] = nc.dram_tensor("d_p00", [128, 2, 512], F32,
                                    kind="ExternalOutput").ap()
        dbg["oT"] = nc.dram_tensor("d_oT", [64, H, OWN], F32,
                                   kind="ExternalOutput").ap()
        dbg["y0"] = nc.dram_tensor("d_y0", [128, C], F32,
                                   kind="ExternalOutput").ap()
        dbg["h2T"] = nc.dram_tensor("d_h2T", [128, CC, OWN], F32,
                                    kind="ExternalOutput").ap()
        dbg["m0"] = nc.dram_tensor("d_m0", [128, OWN], F32,
                                   kind="ExternalOutput").ap()

    # ---------------- DRAM I/O ----------------
    x_d = nc.dram_tensor("xseq", [OWN, C], F32, kind="ExternalInput").ap()
    x2_d = nc.dram_tensor("xseq2", [OWN, C], BF16, kind="ExternalInput").ap()
    wq_d = nc.dram_tensor("wq", [128, KC, 2, C], F8, kind="ExternalInput").ap()
    wk_d = nc.dram_tensor("wk", [128, KC, 2, C], F8, kind="ExternalInput").ap()
    wv_d = nc.dram_tensor("wv", [128, KC, 2, C], F8, kind="ExternalInput").ap()
    wp_d = nc.dram_tensor("wp", [64, CC, 2, C], F8, kind="ExternalInput").ap()
    w1a_d = nc.dram_tensor("w1a", [128, KC, 2, HID], F8, kind="ExternalInput").ap()
    w1b_d = nc.dram_tensor("w1b", [128, KC, 2, HID], F8, kind="ExternalInput").ap()
    w2a_d = nc.dram_tensor("w2a", [128, HKC, 2, C], F8, kind="ExternalInput").ap()
    w2b_d = nc.dram_tensor("w2b", [128, HKC, 2, C], F8, kind="ExternalInput").ap()
    bq_d = nc.dram_tensor("bq", [128, CC], F32, kind="ExternalInput").ap()
    bk_d = nc.dram_tensor("bk", [128, CC], F32, kind="ExternalInput").ap()
    b1_d = nc.dram_tensor("b1e", [128, HC], F32, kind="ExternalInput").ap()
    bias2_d = nc.dram_tensor("bias2", [C], F32, kind="ExternalInput").ap()
    b2_d = nc.dram_tensor("b2t", [C], F32, kind="ExternalInput").ap()
    zr_d = nc.dram_tensor("zr", [8192], F8, kind="ExternalInput").ap()
    out_d = nc.dram_tensor("out", [OWN, C], F32, kind="ExternalOutput").ap()

    def bcast_row(dram_ap, parts=128):
        return bass.AP(tensor=dram_ap.tensor, offset=dram_ap.offset,
                       ap=[[0, parts]] + list(dram_ap.ap))

    with tile.TileContext(nc) as tc:
        # ---------------- persistent constants ----------------
        consts = tc.alloc_tile_pool(name="consts", bufs=1)
        ident = consts.tile([128, 128], BF16, tag="ident")
        make_identity(nc, ident)
        eps_t = consts.tile([128, 1], F32, tag="eps")
        nc.gpsimd.memset(eps_t, EPS)
        if has_bqk:
            bq_sb = consts.tile([128, CC], F32, tag="bq")
            nc.sync.dma_start(out=bq_sb, in_=bq_d)
            bk_sb = consts.tile([128, CC], F32, tag="bk")
            nc.sync.dma_start(out=bk_sb, in_=bk_d)
        if has_b1:
            b1_sb = consts.tile([128, HC], F32, tag="b1")
            nc.sync.dma_start(out=b1_sb, in_=b1_d)
        if has_bias2:
            bias2_sb = consts.tile([128, C], F32, tag="bias2")
            nc.sync.dma_start(out=bias2_sb, in_=bcast_row(bias2_d))
        if has_b2:
            b2_sb = consts.tile([128, C], F32, tag="b2")
            nc.sync.dma_start(out=b2_sb, in_=bcast_row(b2_d))

        # persistent activations
        xres_pool = tc.alloc_tile_pool(name="xres", bufs=1)
        xres = [xres_pool.tile([128, C], F32, tag=f"xr{t}", name=f"xr{t}")
                for t in range(T8)]
        hT_pool = tc.alloc_tile_pool(name="hT", bufs=1)
        hT = hT_pool.tile([128, CC, SEQ], F8, tag="hT", name="hT")

        # QKV weights (prefetch before P1)
        wqkv_pool = tc.alloc_tile_pool(name="wqkv", bufs=1)
        wq_sb = wqkv_pool.tile([128, KC, 2, C], F8, tag="wq", name="wq")
        wk_sb = wqkv_pool.tile([128, KC, 2, C], F8, tag="wk", name="wk")
        wv_sb = wqkv_pool.tile([128, KC, 2, C], F8, tag="wv", name="wv")

        # ============ P1: LN1 + transpose -> hT ============
        p1_ps = tc.alloc_tile_pool(name="p1ps", bufs=4, space="PSUM",
                                   side="right")
        p1_sb = tc.alloc_tile_pool(name="p1sb", bufs=6)
        for t in range(T16):
            if t < T8:
                x_t = xres[t]
                nc.sync.dma_start(out=x_t, in_=x_d[t * 128:(t + 1) * 128, :])
            else:
                x_t = p1_sb.tile([128, C], BF16, tag="xin")
                nc.sync.dma_start(
                    out=x_t, in_=x2_d[(t - T8) * 128:(t - T8 + 1) * 128, :])
            if t == T8:
                nc.sync.dma_start(out=wk_sb, in_=wk_d)
                nc.sync.dma_start(out=wq_sb, in_=wq_d)
            elif t == T8 + 4:
                nc.sync.dma_start(out=wv_sb, in_=wv_d)
            st = p1_sb.tile([128, 2, 6], F32, tag="stats")
            xg = x_t.rearrange("p (n s) -> p n s", s=384)
            for i in range(2):
                nc.vector.bn_stats(out=st[:, i, :], in_=xg[:, i, :])
            mv = p1_sb.tile([128, 2], F32, tag="mv")
            nc.vector.bn_aggr(out=mv, in_=st)
            rstd = p1_sb.tile([128, 1], F32, tag="rstd")
            nc.scalar.activation(out=rstd, in_=mv[:, 1:2], func=AF.Sqrt,
                                 bias=eps_t)
            nc.vector.reciprocal(out=rstd, in_=rstd)
            nmu = p1_sb.tile([128, 1], F32, tag="nmu")
            nc.vector.tensor_scalar(out=nmu, in0=mv[:, 0:1], scalar1=rstd,
                                    scalar2=-1.0, op0=ALU.mult, op1=ALU.mult)
            h8 = p1_sb.tile([128, C], BF16, tag="h8")
            nc.scalar.activation(out=h8, in_=x_t, func=AF.Identity,
                                 bias=nmu, scale=rstd)
            tp = p1_ps.tile([128, CC, 128], BF16, tag="tp")
            for c in range(CC):
                nc.tensor.transpose(tp[:, c, :], h8[:, c * 128:(c + 1) * 128],
                                    ident)
            dst = hT[:, :, t * 128:(t + 1) * 128]
            if t % 2 == 0:
                nc.scalar.copy(out=dst, in_=tp)
            else:
                nc.vector.tensor_copy(out=dst, in_=tp)
        p1_sb.release()
        p1_ps.release()
        if debug_dumps:
            nc.gpsimd.dma_start(out=dbg["hT"], in_=hT)

        # ============ P2: QKV ============
        kv_pool = tc.alloc_tile_pool(name="kv", bufs=1, side="right")
        kT = kv_pool.tile([128, 7, SEQ], F8, tag="kT", name="kT")
        qT = kv_pool.tile([128, 7, OWN], F8, tag="qT", name="qT")
        VW = 80  # fp8 ldweights inner count must be a multiple of 16
        vP = [kv_pool.tile([128, 2, H, VW], F8, tag=f"v{t}", name=f"v{t}")
              for t in range(T16 // 2)]
        # zero strips (strip 6 of kT/qT) via DMA from zero dram
        nc.sync.dma_start(
            out=kT[:, 6, :],
            in_=bass.AP(tensor=zr_d.tensor, offset=zr_d.offset,
                        ap=[[0, 128], [1, SEQ]]))
        nc.sync.dma_start(
            out=qT[:, 6, :],
            in_=bass.AP(tensor=zr_d.tensor, offset=zr_d.offset,
                        ap=[[0, 128], [1, OWN]]))
        for tp_ in range(T16 // 2):
            nc.gpsimd.memset(vP[tp_][:, :, :, DH:DH + 1], 1.0)
            nc.gpsimd.memset(vP[tp_][:, :, :, DH + 1:], 0.0)

        # --- kT (full sequence): out psum [128, 512] per (strip m, nb 4)
        p2k_ps = tc.alloc_tile_pool(name="p2kps", bufs=4, space="PSUM")
        for m in range(CC):
            for half in range(2):
                kps = p2k_ps.tile([128, 1024], F32, tag="kps")
                for r in range(2):
                    nb = half * 2 + r
                    for kc in range(KC):
                        nc.tensor.matmul(
                            kps[:, r * 512:(r + 1) * 512],
                            wk_sb[:, kc, :, m * 128:(m + 1) * 128],
                            hT[:, 2 * kc:2 * kc + 2, nb * 512:(nb + 1) * 512],
                            start=(kc == 0), stop=(kc == KC - 1), perf_mode=DR)
                dst = kT[:, m, half * 1024:(half + 1) * 1024]
                if has_bqk:
                    nc.vector.tensor_scalar(out=dst, in0=kps, scalar1=ISC,
                                            scalar2=bk_sb[:, m:m + 1],
                                            op0=ALU.mult, op1=ALU.add)
                elif (m + half) % 2 == 0:
                    nc.scalar.activation(out=dst, in_=kps, func=AF.Copy,
                                         scale=ISC)
                else:
                    nc.vector.tensor_scalar(out=dst, in0=kps, scalar1=ISC,
                                            scalar2=None, op0=ALU.mult)
            # qT for this strip (own tokens)
            qps = p2k_ps.tile([128, 1024], F32, tag="kps")
            for nb in range(2):
                for kc in range(KC):
                    nc.tensor.matmul(
                        qps[:, nb * 512:(nb + 1) * 512],
                        wq_sb[:, kc, :, m * 128:(m + 1) * 128],
                        hT[:, 2 * kc:2 * kc + 2, nb * 512:(nb + 1) * 512],
                        start=(kc == 0), stop=(kc == KC - 1), perf_mode=DR)
            dst = qT[:, m, :]
            if has_bqk:
                nc.vector.tensor_scalar(out=dst, in0=qps, scalar1=ISC,
                                        scalar2=bq_sb[:, m:m + 1],
                                        op0=ALU.mult, op1=ALU.add)
            else:
                nc.scalar.activation(out=dst, in_=qps, func=AF.Copy,
                                     scale=ISC)
        p2k_ps.release()

        # --- V': out psum [tok 128, 768] per t, copy to vP[t//2] slot t%2
        p2v_ps = tc.alloc_tile_pool(name="p2vps", bufs=3, space="PSUM")
        for t in range(T16):
            vps = p2v_ps.tile([128, 1024], F32, tag="vps")
            for kc in range(KC):
                hsl = hT[:, 2 * kc:2 * kc + 2, t * 128:(t + 1) * 128]
                nc.tensor.matmul(vps[:, 0:512], hsl, wv_sb[:, kc, :, 0:512],
                                 start=(kc == 0), stop=(kc == KC - 1),
                                 perf_mode=DR)
                nc.tensor.matmul(vps[:, 512:768], hsl, wv_sb[:, kc, :, 512:768],
                                 start=(kc == 0), stop=(kc == KC - 1),
                                 perf_mode=DR)
            dst = vP[t // 2][:, t % 2, :, 0:DH]
            vsrc = vps[:, 0:C].rearrange("p (g d) -> p g d", d=DH)
            if t % 2 == 0:
                nc.vector.tensor_scalar(out=dst, in0=vsrc, scalar1=ISC,
                                        scalar2=None, op0=ALU.mult)
            else:
                nc.scalar.activation(out=dst, in_=vsrc, func=AF.Copy,
                                     scale=ISC)
        p2v_ps.release()

        if debug_dumps:
            nc.gpsimd.dma_start(out=dbg["kT"], in_=kT)
            nc.gpsimd.dma_start(out=dbg["qT"], in_=qT)
            nc.gpsimd.dma_start(out=dbg["v0"], in_=vP[0])

        # ============ P3: attention ============
        oT_pool = tc.alloc_tile_pool(name="oT", bufs=1)
        oT = oT_pool.tile([64, H, OWN], F8, tag="oT", name="oT")

        # prefetch proj weights during attention
        wp_pool = tc.alloc_tile_pool(name="wp", bufs=1)
        wp_sb = wp_pool.tile([64, CC, 2, C], F8, tag="wp", name="wp")
        nc.sync.dma_start(out=wp_sb, in_=wp_d)

        s_ps = tc.alloc_tile_pool(name="sps", bufs=3, space="PSUM")
        o_ps = tc.alloc_tile_pool(name="ops", bufs=2, space="PSUM")
        p_pool = tc.alloc_tile_pool(name="pp", bufs=24)
        sm_pool = tc.alloc_tile_pool(name="sm", bufs=4)

        kpitch = kT.ap[0][0]
        qpitch = qT.ap[0][0]
        iters = [(s, hh, qb) for s in range(6) for hh in range(2)
                 for qb in range(2)]
        exp_idx = 0
        pending = []  # (h, qb, ptiles) awaiting O + normalization

        def emit_o_norm(h, qb, ptiles):
            ops_t = o_ps.tile([80, 512], F32, tag="o", name="ops")
            for ktp in range(8):
                nc.tensor.matmul(ops_t, vP[ktp][:, :, h, :], ptiles[ktp],
                                 start=(ktp == 0), stop=(ktp == 7),
                                 perf_mode=DR)
            # normalization: copy -> recip row 64 -> pool bcast/mult
            smf = sm_pool.tile([65, 512], F32, tag="smf", name="smf")
            nc.scalar.copy(out=smf, in_=ops_t[0:65, :])
            rec = sm_pool.tile([1, 512], F32, tag="rec", name="rec")
            nc.vector.reciprocal(out=rec, in_=smf[64:65, :])
            bcast = sm_pool.tile([64, 512], F32, tag="bc", name="bc")
            nc.gpsimd.partition_broadcast(bcast, rec[0:1, :])
            nc.gpsimd.tensor_mul(oT[:, h, qb * 512:(qb + 1) * 512],
                                 smf[0:64, :], bcast)

        for s, hh, qb in iters:
            h = 2 * s + hh
            pb = hh * 64  # partition base for this head in kT/qT
            ptiles = []
            for ktp in range(8):
                spair = s_ps.tile([128, 1024], F32, tag="spair",
                                  name="spair")
                for j in range(2):
                    kt = 2 * ktp + j
                    lhsT = bass.AP(
                        tensor=kT.tensor,
                        offset=kT.offset + pb * kpitch + s * SEQ + kt * 128,
                        ap=[[kpitch, 64], [(6 - s) * SEQ, 2], [1, 128]])
                    rhs = bass.AP(
                        tensor=qT.tensor,
                        offset=qT.offset + pb * qpitch + s * OWN + qb * 512,
                        ap=[[qpitch, 64], [(6 - s) * OWN, 2], [1, 512]])
                    nc.tensor.matmul(
                        spair[:, j * 512:(j + 1) * 512], lhsT, rhs,
                        start=True, stop=True, perf_mode=DR)
                ptile = p_pool.tile([128, 2, 512], F8, tag="pt", name="pt")
                if (exp_idx * ACT_OF_16) % 16 < ACT_OF_16:
                    nc.scalar.activation(out=ptile, in_=spair,
                                         func=AF.Exp, scale=SM_SCALE)
                else:
                    nc.vector.tensor_scalar(
                        out=ptile.bitcast(I8), in0=spair,
                        scalar1=float(EXP_A), scalar2=float(EXP_B),
                        op0=ALU.mult, op1=ALU.add)
                exp_idx += 1
                ptiles.append(ptile)
            if debug_dumps and h == 0 and qb == 0:
                nc.gpsimd.dma_start(out=dbg["p00"], in_=ptiles[0])
            pending.append((h, qb, ptiles))
            if len(pending) > PIPE_LAG:
                emit_o_norm(*pending.pop(0))
        while pending:
            emit_o_norm(*pending.pop(0))
        sm_pool.release()
        p_pool.release()
        o_ps.release()
        s_ps.release()
        kv_pool.release()
        if debug_dumps:
            nc.gpsimd.dma_start(out=dbg["oT"], in_=oT)

        # ============ P4: proj + residual + LN2 + h2T ============
        y_pool = tc.alloc_tile_pool(name="y", bufs=1, side="right")
        y = [y_pool.tile([128, C], F32, tag=f"y{t}", name=f"y{t}")
             for t in range(T8)]
        h2T_pool = tc.alloc_tile_pool(name="h2T", bufs=1, side="right")
        h2T = h2T_pool.tile([128, CC, OWN], F8, tag="h2T", name="h2T")
        # prefetch MLP-up weight pairs
        w1_pool = tc.alloc_tile_pool(name="w1p", bufs=1, side="right")
        w1a_sb = w1_pool.tile([128, KC, 2, HID], F8, tag="w1a", name="w1a")
        nc.sync.dma_start(out=w1a_sb, in_=w1a_d)
        if W1_PAIR:
            w1b_sb = w1_pool.tile([128, KC, 2, HID], F8, tag="w1b", name="w1b")
            nc.sync.dma_start(out=w1b_sb, in_=w1b_d)
        else:
            w1b_sb = None

        p4_ps = tc.alloc_tile_pool(name="p4ps", bufs=2, space="PSUM")
        p4t_ps = tc.alloc_tile_pool(name="p4tps", bufs=4, space="PSUM")
        p4_sb = tc.alloc_tile_pool(name="p4sb", bufs=3)
        for t in range(T8):
            aps = p4_ps.tile([128, 1024], F32, tag="aps")
            for sp in range(CC):
                lhsT = oT[:, 2 * sp:2 * sp + 2, t * 128:(t + 1) * 128]
                nc.tensor.matmul(aps[:, 0:512], lhsT, wp_sb[:, sp, :, 0:512],
                                 start=(sp == 0), stop=(sp == CC - 1),
                                 perf_mode=DR)
                nc.tensor.matmul(aps[:, 512:768], lhsT, wp_sb[:, sp, :, 512:768],
                                 start=(sp == 0), stop=(sp == CC - 1),
                                 perf_mode=DR)
            nc.vector.scalar_tensor_tensor(
                out=y[t], in0=aps[:, 0:C], scalar=ISC, in1=xres[t],
                op0=ALU.mult, op1=ALU.add)
            if has_bias2:
                nc.vector.tensor_add(y[t], y[t], bias2_sb)
            st = p4_sb.tile([128, 2, 6], F32, tag="stats")
            yg = y[t].rearrange("p (n s) -> p n s", s=384)
            for i in range(2):
                nc.vector.bn_stats(out=st[:, i, :], in_=yg[:, i, :])
            mv = p4_sb.tile([128, 2], F32, tag="mv")
            nc.vector.bn_aggr(out=mv, in_=st)
            rstd = p4_sb.tile([128, 1], F32, tag="rstd")
            nc.scalar.activation(out=rstd, in_=mv[:, 1:2], func=AF.Sqrt,
                                 bias=eps_t)
            nc.vector.reciprocal(out=rstd, in_=rstd)
            nmu = p4_sb.tile([128, 1], F32, tag="nmu")
            nc.vector.tensor_scalar(out=nmu, in0=mv[:, 0:1], scalar1=rstd,
                                    scalar2=-1.0, op0=ALU.mult, op1=ALU.mult)
            h2 = p4_sb.tile([128, C], BF16, tag="h2")
            nc.scalar.activation(out=h2, in_=y[t], func=AF.Identity,
                                 bias=nmu, scale=rstd)
            tp = p4t_ps.tile([128, CC, 128], BF16, tag="tp")
            for c in range(CC):
                nc.tensor.transpose(tp[:, c, :], h2[:, c * 128:(c + 1) * 128],
                                    ident)
            dst = h2T[:, :, t * 128:(t + 1) * 128]
            if t % 2 == 0:
                nc.scalar.copy(out=dst, in_=tp)
            else:
                nc.vector.tensor_copy(out=dst, in_=tp)
        if debug_dumps:
            nc.gpsimd.dma_start(out=dbg["y0"], in_=y[0])
            nc.gpsimd.dma_start(out=dbg["h2T"], in_=h2T)
        p4_sb.release()
        p4t_ps.release()
        p4_ps.release()
        wp_pool.release()
        oT_pool.release()
        wqkv_pool.release()
        hT_pool.release()
        xres_pool.release()

        # ============ P5: MLP up + gelu ============
        m_pool = tc.alloc_tile_pool(name="m", bufs=1)
        mT = m_pool.tile([128, HC, OWN], F8, tag="mT", name="mT")
        w2_pool = tc.alloc_tile_pool(name="w2p", bufs=1)
        w2a_sb = w2_pool.tile([128, HKC, 2, C], F8, tag="w2a", name="w2a")
        nc.sync.dma_start(out=w2a_sb, in_=w2a_d)
        if W2_PAIR:
            w2b_sb = w2_pool.tile([128, HKC, 2, C], F8, tag="w2b", name="w2b")
            nc.sync.dma_start(out=w2b_sb, in_=w2b_d)
        else:
            w2b_sb = None

        p5_ps = tc.alloc_tile_pool(name="p5ps", bufs=3, space="PSUM")
        for m in range(HC):
            hps = p5_ps.tile([128, 1024], F32, tag="hps")
            for nb in range(2):
                first = True
                for wsl in ((w1a_sb, w1b_sb) if W1_PAIR else (w1a_sb,)):
                    for kc in range(KC):
                        nc.tensor.matmul(
                            hps[:, nb * 512:(nb + 1) * 512],
                            wsl[:, kc, :, m * 128:(m + 1) * 128],
                            h2T[:, 2 * kc:2 * kc + 2,
                                nb * 512:(nb + 1) * 512],
                            start=first,
                            stop=(wsl is (w1b_sb if W1_PAIR else w1a_sb)
                                  and kc == KC - 1),
                            perf_mode=DR)
                        first = False
            kw = dict(bias=b1_sb[:, m:m + 1]) if has_b1 else {}
            nc.scalar.activation(out=mT[:, m, :], in_=hps, func=AF.Gelu,
                                 scale=ISC, **kw)
        if debug_dumps:
            nc.gpsimd.dma_start(out=dbg["m0"], in_=mT[:, 0, :])
        p5_ps.release()
        w1_pool.release()
        h2T_pool.release()

        # ============ P6: MLP down + final residual ============
        p6_ps = tc.alloc_tile_pool(name="p6ps", bufs=4, space="PSUM")
        p6_sb = tc.alloc_tile_pool(name="p6sb", bufs=3)
        for t in range(T8):
            mps = p6_ps.tile([128, 1024], F32, tag="mps")
            first = True
            for wsl in ((w2a_sb, w2b_sb) if W2_PAIR else (w2a_sb,)):
                for kc in range(HKC):
                    lhsT = mT[:, 2 * kc:2 * kc + 2, t * 128:(t + 1) * 128]
                    last = (wsl is (w2b_sb if W2_PAIR else w2a_sb)
                            and kc == HKC - 1)
                    nc.tensor.matmul(mps[:, 0:512], lhsT, wsl[:, kc, :, 0:512],
                                     start=first, stop=last, perf_mode=DR)
                    nc.tensor.matmul(mps[:, 512:768], lhsT,
                                     wsl[:, kc, :, 512:768],
                                     start=first, stop=last, perf_mode=DR)
                    first = False
            o_t = p6_sb.tile([128, C], F32, tag="out")
            nc.vector.scalar_tensor_tensor(
                out=o_t, in0=mps[:, 0:C], scalar=ISC, in1=y[t],
                op0=ALU.mult, op1=ALU.add)
            if has_b2:
                nc.vector.tensor_add(o_t, o_t, b2_sb)
            nc.sync.dma_start(out=out_d[t * 128:(t + 1) * 128, :], in_=o_t)
        p6_ps.release()
        p6_sb.release()
        w2_pool.release()
        m_pool.release()
        y_pool.release()
        consts.release()

    nc.compile()
    return nc


def _f8(x):
    return np.asarray(x, dtype=E4)


def build_in_maps(x, ln1_g, ln1_b, w_qkv, w_proj, b_proj, ln2_g, ln2_b,
                  w1, b1, w2, b2):
    x = np.asarray(x, np.float32)
    ln1_g = np.asarray(ln1_g, np.float32)
    ln1_b = np.asarray(ln1_b, np.float32)
    w_qkv = np.asarray(w_qkv, np.float32)
    w_proj = np.asarray(w_proj, np.float32)
    b_proj = np.asarray(b_proj, np.float32)
    ln2_g = np.asarray(ln2_g, np.float32)
    ln2_b = np.asarray(ln2_b, np.float32)
    w1 = np.asarray(w1, np.float32)
    b1 = np.asarray(b1, np.float32)
    w2 = np.asarray(w2, np.float32)
    b2 = np.asarray(b2, np.float32)

    wqkv_eff = w_qkv * ln1_g[:, None]
    bqkv = ln1_b @ w_qkv
    bq = bqkv[0:C]
    bk = bqkv[C:2 * C]
    bv = bqkv[2 * C:3 * C]
    bias2 = bv @ w_proj + b_proj
    w1_eff = w1 * ln2_g[:, None]
    b1_eff = b1 + ln2_b @ w1

    def dr_slabs(w, pair):
        # w [Cin, M] -> [128, Cin//256, 2, M] with [p, kc, j, m] =
        # w[kc*256 + j*128 + p, m], scaled by WS, fp8 (+ residual slab)
        cin = w.shape[0]
        r = (w * WS).reshape(cin // 256, 2, 128, w.shape[1]).transpose(2, 0, 1, 3)
        a = _f8(r)
        if not pair:
            return np.ascontiguousarray(a), None
        b_ = _f8(r - a.astype(np.float32))
        return np.ascontiguousarray(a), np.ascontiguousarray(b_)

    wq8, _ = dr_slabs(wqkv_eff[:, 0:C], False)
    wk8, _ = dr_slabs(wqkv_eff[:, C:2 * C], False)
    wv8, _ = dr_slabs(wqkv_eff[:, 2 * C:3 * C], False)
    w1a, w1b = dr_slabs(w1_eff, True)
    w2a, w2b = dr_slabs(w2, True)
    # wp: [64 d, CC strip-pairs s, 2 j, C] with [d, s, j, m] =
    # w_proj[(2s + j)*64 + d, m] scaled
    wp8 = _f8((w_proj * WS).reshape(CC, 2, 64, C).transpose(2, 0, 1, 3))
    wp8 = np.ascontiguousarray(wp8)

    def col128(v):
        # [C or HID] -> [128, nchunk] with [p, m] = v[m*128 + p]
        nchunk = v.shape[0] // 128
        return np.ascontiguousarray(v.reshape(nchunk, 128).T)

    has_bqk = bool(np.any(bq != 0) or np.any(bk != 0))
    has_bias2 = bool(np.any(bias2 != 0))
    has_b2 = bool(np.any(b2 != 0))
    has_b1 = bool(np.any(b1_eff != 0))

    common = {
        "wq": wq8, "wk": wk8, "wv": wv8, "wp": wp8,
        "w1a": w1a, "w1b": w1b, "w2a": w2a, "w2b": w2b,
        "bq": col128(bq), "bk": col128(bk), "b1e": col128(b1_eff),
        "bias2": bias2.astype(np.float32), "b2t": b2,
        "zr": np.zeros(8192, E4),
    }
    in_maps = []
    for c in range(NCORES):
        b_i, half = divmod(c, 2)
        if half == 0:
            xseq = x[b_i]
        else:
            xseq = np.concatenate([x[b_i][OWN:], x[b_i][:OWN]], axis=0)
        m = dict(common)
        m["xseq"] = np.ascontiguousarray(xseq[:OWN])
        m["xseq2"] = np.ascontiguousarray(
            xseq[OWN:].astype(ml_dtypes.bfloat16))
        in_maps.append(m)
    return in_maps, (has_bqk, has_bias2, has_b2, has_b1)


def kernel(**inputs):
    in_maps, key = build_in_maps(**inputs)
    if key not in _CACHE:
        _CACHE[key] = _build_program(key)
    nc = _CACHE[key]
    res = run_bass_kernel_spmd(nc, in_maps, core_ids=list(range(NCORES)))
    out = np.empty((B, N, C), np.float32)
    for c in range(NCORES):
        b_i, half = divmod(c, 2)
        out[b_i, half * OWN:(half + 1) * OWN, :] = res.results[c]["out"]
    return out

